# revision 1
# baseline (speedup 1.0000x reference)
"""GNN message-passing kernel for 8 Trainium2 NeuronCores (Bass/Tile).

Sharding: each core owns 2500 nodes + all edges targeting them. Node state
is feature-major in SBUF; after each GRU update it is written node-major
(bf16, 128-wide rows, partition-major "p j f" layout) to DRAM and
AllGathered so any core can dma_gather arbitrary source rows.  Per-edge
weights are never materialized:
  msg_e = h[src_e] @ (ea_e @ nnW^T).reshape(D,D)
is computed as  Z[(k,i),e] = ea[k,e] * h[i,src_e];  msg = G^T @ Z
with G a host-prepacked rearrangement of nnW.  1/deg is folded exactly
into the edge attributes on the host (rrelu is positively homogeneous, the
encoder bias rides along as an extra attr row).  segment-sum over targets
is a matmul against host-built 0/1 staircase blocks (edges sorted by
target, each 512-node tile padded to an integral number of 128-edge
chunks).
"""
import sys, os
sys.path.insert(0, "/opt/trn_rl_repo")
import numpy as np
import ml_dtypes

ABL = set(os.environ.get("KABL", "").split(","))  # ablation flags

import concourse.bass as bass
import concourse.bacc as bacc
import concourse.mybir as mybir
import concourse.tile as tile
from concourse.bass_utils import run_bass_kernel_spmd

F32 = mybir.dt.float32
BF16 = mybir.dt.bfloat16
I16 = mybir.dt.int16
AF = mybir.ActivationFunctionType
ALU = mybir.AluOpType

N, E, E3, D = 20000, 30000, 4000, 64
D2 = 2 * D
C = 8
NL = N // C          # nodes per core (2500)
NLP = 2560           # padded rows per core in gathered state (mult of 128)
NJ = NLP // 128      # 20 row-chunks per core
NT = 512             # node-tile / matmul moving chunk
NTC = (NL + NT - 1) // NT
SLOPE = (1.0 / 8.0 + 1.0 / 3.0) / 2.0
EPS = 1e-5
EF = 512             # final readout edges per core (500 real)

bfd = ml_dtypes.bfloat16


# ------------------------------------------------------- packed const layout

def _cf32_layout():
    ents = [("bln", 64, 1),
            ("c1b", 64, 1), ("c2b", 128, 1),
            ("br1", 64, 1), ("bz1", 64, 1), ("bin1", 64, 1), ("bhn1", 64, 1),
            ("br2", 128, 1), ("bz2", 128, 1), ("bin2", 128, 1),
            ("bhn2", 128, 1), ("b1c", 128, 1), ("b2c", 128, 1),
            ("lwWt", 8, 384), ("lbp", 8, 1), ("ea3locT", 8, EF)]
    lay, c = {}, 0
    for name, p, w in ents:
        lay[name] = (p, c, w)
        c += w
    return lay, c


def _cbf_layout():
    ents = [("G1", 128, 384), ("REP1", 12, 768), ("G2", 128, 1024),
            ("REP2", 8, 1024), ("wih1", 64, 192), ("whh1", 64, 192),
            ("wih2", 128, 384), ("whh2", 128, 384),
            ("W1cT", 64, 128), ("W2cT", 128, 128),
            ("Wln", 8, 64), ("leWt", 20, 12), ("eye", 128, 128),
            ("eyed", 64, 128)]
    lay, c = {}, 0
    for name, p, w in ents:
        lay[name] = (p, c, w)
        c += w
    return lay, c


def _ci16_layout(EP1, EP3):
    ents = [("gidx1", 128, EP1 // 16), ("gidx3", 128, EP3 // 16),
            ("gf0", 128, EF // 16), ("gf1", 128, EF // 16)]
    lay, c = {}, 0
    for name, p, w in ents:
        lay[name] = (p, c, w)
        c += w
    return lay, c


# ----------------------------------------------------------------- host prep

def _wrap16(idx):
    n = len(idx)
    w = idx.reshape(n // 16, 16).T.astype(np.int16)
    return np.tile(w, (8, 1)).copy()


def _pad_id(n):
    """global node id -> row id in gathered state ("p j f" layout)."""
    c = n // NL
    r = n % NL
    return c * NLP + (r % 128) * NJ + r // 128


def _affine_bn(g, be, m, v):
    a = g / np.sqrt(v + EPS)
    return a, be - m * a


def _prep_edges(src, tgt, attr, n_attr):
    """attr must already carry any per-edge scaling (1/deg)."""
    owner = tgt // NL
    per_core = []
    maxrun = 1
    for c in range(C):
        sel = np.where(owner == c)[0]
        tl = tgt[sel] - c * NL
        order = np.argsort(tl, kind="stable")
        sel, tl = sel[order], tl[order]
        per_core.append((sel, tl))
        for t in range(NTC):
            maxrun = max(maxrun, int(((tl // NT) == t).sum()))
    cpt = (maxrun + 127) // 128
    ep = NTC * cpt * 128

    gidx = np.zeros((C, ep), np.int64)
    eaT = np.zeros((C, n_attr, ep), bfd)
    s_blocks = np.zeros((C, 128, NTC * cpt * NT), bfd)  # "p (j t)" layout

    for c, (sel, tl) in enumerate(per_core):
        for t in range(NTC):
            msk = (tl // NT) == t
            idxs, tls = sel[msk], tl[msk]
            k = len(idxs)
            pos = t * cpt * 128
            gidx[c, pos:pos + k] = _pad_id(src[idxs])
            eaT[c, :, pos:pos + k] = attr[idxs].T.astype(bfd)
            rel = tls - t * NT
            ar = np.arange(k) + pos
            s_blocks[c, ar % 128, (ar // 128) * NT + rel] = 1.0
    return dict(ep=ep, gidx=gidx, eaT=eaT, s=s_blocks)


def _host_prep(inp):
    g = lambda k: np.asarray(inp[k], np.float32)
    ei = np.asarray(inp["edge_index"], np.int64)
    ei3 = np.asarray(inp["edge_index3"], np.int64)

    a, b = _affine_bn(g("nx_g"), g("nx_be"), g("nx_m"), g("nx_v"))
    Wln = (a[:, None] * g("ln_W").T).astype(np.float32)
    bln = (b @ g("ln_W").T + g("ln_b")).astype(np.float32)

    # stage1: fold 1/deg into the edge attrs (rrelu is pos. homogeneous);
    # the encoder bias le_b rides along as a 20th attr row (=1/deg).
    deg1 = np.maximum(np.bincount(ei[1], minlength=N), 1).astype(np.float64)
    inv1 = (1.0 / deg1[ei[1]]).astype(np.float32)
    attr1 = np.concatenate(
        [g("edge_attr") * inv1[:, None], inv1[:, None]], axis=1)  # (E, 20)
    leWt20 = np.concatenate(
        [g("le_W").T, g("le_b").reshape(1, -1)], axis=0)          # (20, 12)
    e1 = _prep_edges(ei[0], ei[1], attr1, 20)

    nn1 = g("nn1_W")
    G1 = np.zeros((128, 6, 64), np.float32)           # partition-first
    REP1 = np.zeros((12, 6, 128), np.float32)
    for cc in range(6):
        for half, k in enumerate((2 * cc, 2 * cc + 1)):
            G1[half * 64:(half + 1) * 64, cc, :] = nn1[:, k].reshape(64, 64)
            REP1[k, cc, half * 64:(half + 1) * 64] = 1.0

    src3 = np.concatenate([ei3[0], ei3[1]])
    tgt3 = np.concatenate([ei3[1], ei3[0]])
    deg3 = np.maximum(np.bincount(tgt3, minlength=N), 1).astype(np.float64)
    inv3 = (1.0 / deg3[tgt3]).astype(np.float32)
    attr3 = np.concatenate([g("edge_attr3"), g("edge_attr3")], axis=0)
    attr3 = attr3 * inv3[:, None]
    e2 = _prep_edges(src3, tgt3, attr3, 8)
    nn2 = g("nn2_W")
    G2 = np.zeros((128, 8, 128), np.float32)
    REP2 = np.zeros((8, 8, 128), np.float32)
    for k in range(8):
        G2[:, k, :] = nn2[:, k].reshape(D2, D2)
        REP2[k, k, :] = 1.0

    f_i0 = np.zeros((C, EF), np.int64)
    f_i1 = np.zeros((C, EF), np.int64)
    ea3locT = np.zeros((C, 8, EF), np.float32)
    npc = E3 // C
    for c in range(C):
        lo = c * npc
        f_i0[c, :npc] = _pad_id(ei3[0, lo:lo + npc])
        f_i1[c, :npc] = _pad_id(ei3[1, lo:lo + npc])
        ea3locT[c, :, :npc] = g("edge_attr3")[lo:lo + npc].T

    a_nm, b_nm = _affine_bn(g("nm_g"), g("nm_be"), g("nm_m"), g("nm_v"))
    a_nm = a_nm.copy()
    a_nm[0:D2] *= 0.5
    lwWt = (g("lw_W") * a_nm[:, None]).T.astype(np.float32)   # (8,384)
    lbp = (g("lb_W")[0] + b_nm @ g("lw_W")).astype(np.float32)

    alc, blc = _affine_bn(g("lc_g"), g("lc_be"), g("lc_m"), g("lc_v"))
    W1c = g("lc_w1") * alc[None, :]
    b1c = (g("lc_w1") @ blc + g("lc_b1")).astype(np.float32)

    bih1, bhh1 = g("g1_bih"), g("g1_bhh")
    bih2, bhh2 = g("g2_bih"), g("g2_bhh")

    EP1, EP3 = e1["ep"], e2["ep"]
    f32lay, f32w = _cf32_layout()
    bflay, bfw = _cbf_layout()
    i16lay, i16w = _ci16_layout(EP1, EP3)

    f32c = {
        "bln": bln.reshape(-1, 1),
        "c1b": g("c1_b").reshape(-1, 1), "c2b": g("c2_b").reshape(-1, 1),
        "br1": (bih1 + bhh1)[0:D].reshape(-1, 1),
        "bz1": (bih1 + bhh1)[D:2 * D].reshape(-1, 1),
        "bin1": bih1[2 * D:].reshape(-1, 1),
        "bhn1": bhh1[2 * D:].reshape(-1, 1),
        "br2": (bih2 + bhh2)[0:D2].reshape(-1, 1),
        "bz2": (bih2 + bhh2)[D2:2 * D2].reshape(-1, 1),
        "bin2": bih2[2 * D2:].reshape(-1, 1),
        "bhn2": bhh2[2 * D2:].reshape(-1, 1),
        "b1c": b1c.reshape(-1, 1), "b2c": g("lc_b2").reshape(-1, 1),
        "lwWt": lwWt, "lbp": lbp.reshape(-1, 1),
    }
    bfc = {
        "G1": G1.reshape(128, 384), "REP1": REP1.reshape(12, 768),
        "G2": G2.reshape(128, 1024), "REP2": REP2.reshape(8, 1024),
        "wih1": g("g1_wih").T, "whh1": g("g1_whh").T,
        "wih2": g("g2_wih").T, "whh2": g("g2_whh").T,
        "W1cT": W1c.T, "W2cT": g("lc_w2").T,
        "Wln": Wln, "leWt": leWt20, "eye": np.eye(128, dtype=np.float32),
        "eyed": np.concatenate([np.eye(64), np.eye(64)], axis=1),
    }

    CF = np.zeros((128, f32w), np.float32)
    for name, arr in f32c.items():
        p, c0, w = f32lay[name]
        CF[0:p, c0:c0 + w] = arr
    CB = np.zeros((128, bfw), bfd)
    for name, arr in bfc.items():
        p, c0, w = bflay[name]
        CB[0:p, c0:c0 + w] = arr.astype(bfd)

    xs = g("x")
    in_maps = []
    for c in range(C):
        CFc = CF.copy()
        p, c0, w = f32lay["ea3locT"]
        CFc[0:p, c0:c0 + w] = ea3locT[c]
        CI = np.zeros((128, i16w), np.int16)
        for name, arr in (("gidx1", _wrap16(e1["gidx"][c])),
                          ("gidx3", _wrap16(e2["gidx"][c])),
                          ("gf0", _wrap16(f_i0[c])),
                          ("gf1", _wrap16(f_i1[c]))):
            p, c0, w = i16lay[name]
            CI[0:p, c0:c0 + w] = arr
        m = {
            "xT": xs[c * NL:(c + 1) * NL].T.astype(bfd),
            "eaT1": e1["eaT"][c],
            "eaT3": e2["eaT"][c],
            "S1": e1["s"][c],
            "S3": e2["s"][c],
            "CF": CFc, "CB": CB, "CI": CI,
        }
        in_maps.append({k: np.ascontiguousarray(v) for k, v in m.items()})
    static = (EP1, EP3)
    return static, in_maps


# ------------------------------------------------------------- kernel builder

def _build(EP1, EP3, reps=1):
    nc = bacc.Bacc("TRN2", target_bir_lowering=False, debug=False,
                   num_devices=C)
    J1, J3 = EP1 // 128, EP3 // 128
    LZ = max(6 * (EP1 - (EP1 // 256) * 128), 8 * EP3)  # z arena (max phase)
    LE = max(6 * EP1, 8 * EP3)            # eax arena
    LG = max(EP1, EP3)
    LM = max(J1 * 64, J3 * 128)           # msg_em arena

    f32lay, f32w = _cf32_layout()
    bflay, bfw = _cbf_layout()
    i16lay, i16w = _ci16_layout(EP1, EP3)

    def inp(name, shape, dt=F32):
        return nc.dram_tensor(name, list(shape), dt, kind="ExternalInput")

    xT = inp("xT", (8, NL), BF16)
    eaT1 = inp("eaT1", (20, EP1), BF16)
    eaT3 = inp("eaT3", (8, EP3), BF16)
    S1 = inp("S1", (128, J1 * NT), BF16)
    S3 = inp("S3", (128, J3 * NT), BF16)
    CFd = inp("CF", (128, f32w))
    CBd = inp("CB", (128, bfw), BF16)
    CId = inp("CI", (128, i16w), I16)
    out_f = nc.dram_tensor("out_f", [1, EF], F32, kind="ExternalOutput")

    with tile.TileContext(nc) as tc:
        with (
            tc.tile_pool(name="cst", bufs=1) as cp,
            tc.tile_pool(name="arena", bufs=1) as ar,
            tc.tile_pool(name="wk", bufs=2) as wp,
            tc.tile_pool(name="sblk", bufs=3) as sp,
            tc.tile_pool(name="fin", bufs=1) as fp,
            tc.tile_pool(name="ps2", bufs=2, space="PSUM") as p2,
            tc.tile_pool(name="ps1", bufs=1, space="PSUM") as p1,
            tc.tile_pool(name="ptp", bufs=2, space="PSUM") as pp,
            tc.tile_pool(name="dram", bufs=1, space="DRAM") as dp,
        ):
            CFt = cp.tile([128, f32w], F32, tag="CFt")
            nc.sync.dma_start(CFt[:], CFd[:])
            CBt = cp.tile([128, bfw], BF16, tag="CBt")
            nc.sync.dma_start(CBt[:], CBd[:])
            CIt = cp.tile([128, i16w], I16, tag="CIt")
            nc.sync.dma_start(CIt[:], CId[:])

            def cf(name, sub=None, subw=None):
                p, c0, w = f32lay[name]
                if sub is not None:
                    return CFt[0:p, c0 + sub:c0 + sub + subw]
                return CFt[0:p, c0:c0 + w]

            def cb(name, sub=None, subw=None):
                p, c0, w = bflay[name]
                if sub is not None:
                    return CBt[0:p, c0 + sub:c0 + sub + subw]
                return CBt[0:p, c0:c0 + w]

            def ci(name):
                p, c0, w = i16lay[name]
                return CIt[0:p, c0:c0 + w]

            c_ones = cp.tile([128, 1], F32, tag="cones")
            nc.vector.memset(c_ones[:], 1.0)

            # arenas (slot-shared across stages)
            A_eax = ar.tile([128, LE], BF16, tag="eax")
            A_z = ar.tile([128, LZ], BF16, tag="z")
            A_g = ar.tile([128, 1, LG], BF16, tag="g")
            A_me = ar.tile([128, LM], BF16, tag="me")
            A_hb = ar.tile([128, NL], BF16, tag="hb")
            A_m = ar.tile([128, NL], BF16, tag="m")
            A_hn = ar.tile([128, NJ, 128], BF16, tag="hx")
            A_s3 = ar.tile([128, J3 * NT], BF16, tag="s3")

            H1_loc = dp.tile([128, NJ, 128], BF16)
            H2_loc = dp.tile([128, NJ, 128], BF16)

            def mov_chunks(n):
                return [(j, slice(j * NT, min((j + 1) * NT, n)),
                         min((j + 1) * NT, n) - j * NT)
                        for j in range((n + NT - 1) // NT)]

            def lrelu_act(out, in_, bias):
                """out = rrelu(in_ + bias) = max(t, SLOPE*t)."""
                P, Fr = out.shape[0], out.shape[-1]
                t = wp.tile([128, NT], BF16, tag="lrt")
                nc.scalar.activation(t[0:P, 0:Fr], in_, AF.Identity, bias=bias)
                nc.vector.scalar_tensor_tensor(
                    out, t[0:P, 0:Fr], SLOPE, t[0:P, 0:Fr],
                    op0=ALU.mult, op1=ALU.max)

            def one_pass():
              H1a = dp.tile([C * NLP, 128], BF16, addr_space="Shared", name="H1a")
              H1b = dp.tile([C * NLP, 128], BF16, addr_space="Shared", name="H1b")
              H2a = dp.tile([C * NLP, 128], BF16, addr_space="Shared", name="H2a")
              H2b = dp.tile([C * NLP, 128], BF16, addr_space="Shared", name="H2b")
              H2c = dp.tile([C * NLP, 128], BF16, addr_space="Shared", name="H2c")
              # ---------------- encode
              sc = nc.named_scope
              with sc("encode"):
                xa = wp.tile([8, NL], BF16, tag="xa", bufs=1)
                nc.sync.dma_start(xa[:], xT[:])
                for j, sl, w in mov_chunks(NL):
                  p = p2.tile([64, NT], F32, tag="p512")
                  nc.tensor.matmul(p[:, 0:w], cb("Wln"), xa[:, sl],
                                   start=True, stop=True)
                  lrelu_act(A_hb[0:64, sl], p[:, 0:w], cf("bln"))

              def export_chunk(feat, hb, j):
                  lo, hi = j * 128, min((j + 1) * 128, NL)
                  w = hi - lo
                  if w < 128:
                      src = wp.tile([feat, 128], BF16, tag="exs")
                      nc.vector.memset(src[:], 0.0)
                      nc.vector.tensor_copy(src[0:feat, 0:w],
                                            hb[0:feat, lo:hi])
                      sfrom = src[0:feat, :]
                  else:
                      sfrom = hb[0:feat, lo:hi]
                  pt = pp.tile([128, 128], BF16, tag="ptp")
                  if feat == 64:
                      nc.tensor.transpose(pt[:], sfrom, cb("eyed")[0:64, :])
                  else:
                      nc.tensor.transpose(pt[:], sfrom, cb("eye"))
                  nc.scalar.activation(A_hn[:, j, :], pt[:], AF.Identity)

              def export_fin(H_loc, H):
                  nc.sync.dma_start(H_loc[:], A_hn[:])
                  nc.gpsimd.collective_compute(
                      "AllGather", ALU.bypass, replica_groups=[list(range(C))],
                      ins=[H_loc[:].opt()], outs=[H[:].opt()])

              def export_state(feat, H_loc, H, hb, tag="exp"):
                  """feature-major bf16 -> node-major "p j f" rows + AllGather."""
                  with sc(tag):
                    for j in range(NJ):
                        export_chunk(feat, hb, j)
                    export_fin(H_loc, H)

              def msg_pass(H, gi, EP, nk, Gc, S_sb, S_d, J, feat, m_out, cbias,
                           tag="mp"):
                with sc(tag):
                  npass = 2 if feat == 64 else 1
                  jb = [0] + ([(J // 2) * 128, EP] if npass == 2 else [EP])
                  for lo, hi in zip(jb[:-1], jb[1:]):
                      gw = hi - lo
                      nc.gpsimd.dma_gather(
                          A_g[:, :, lo:hi], H[:],
                          gi[:, lo // 16:hi // 16], gw, gw, 128,
                          transpose=True, single_packet=False)
                  g2 = A_g[:].rearrange("p one e -> p (one e)")
                  for ph in range(npass):
                      base, eph = jb[ph], jb[ph + 1] - jb[ph]
                      for kc in range(nk):
                          nc.vector.tensor_tensor(
                              A_z[:, kc * eph:(kc + 1) * eph],
                              g2[:, base:base + eph],
                              A_eax[:, kc * EP + base:kc * EP + base + eph],
                              op=ALU.mult)
                      for j, sl, w in mov_chunks(eph):
                          p = p2.tile([feat, NT], F32, tag="p512")
                          for kc in range(nk):
                              nc.tensor.matmul(
                                  p[:, 0:w], Gc(kc, feat),
                                  A_z[:, kc * eph + sl.start:kc * eph + sl.stop],
                                  start=(kc == 0), stop=(kc == nk - 1))
                          mc = wp.tile([feat, NT], BF16, tag="mc")
                          nc.vector.tensor_copy(mc[:, 0:w], p[:, 0:w])
                          for q in range(w // 128):
                              jj = (base + sl.start) // 128 + q
                              pt = pp.tile([128, 128], BF16, tag="ptp")
                              nc.tensor.transpose(
                                  pt[:, 0:feat],
                                  mc[0:feat, q * 128:(q + 1) * 128],
                                  cb("eye", 0, feat)[0:feat, :])
                              nc.vector.tensor_copy(
                                  A_me[:, jj * feat:(jj + 1) * feat],
                                  pt[:, 0:feat])
                  cpt = J // NTC
                  for t in range(NTC):
                      pm = p2.tile([feat, NT], F32, tag="p512")
                      if S_sb is None:
                          sbt = sp.tile([128, cpt * NT], BF16, tag="Sblk")
                          nc.sync.dma_start(
                              sbt[:], S_d[:, t * cpt * NT:(t + 1) * cpt * NT])
                      for q in range(cpt):
                          j = t * cpt + q
                          if S_sb is not None:
                              sb = S_sb[:, j * NT:(j + 1) * NT]
                          else:
                              sb = sbt[:, q * NT:(q + 1) * NT]
                          nc.tensor.matmul(pm[:], A_me[:, j * feat:(j + 1) * feat],
                                           sb, start=(q == 0),
                                           stop=(q == cpt - 1))
                      hi = min(NT, NL - t * NT)
                      lrelu_act(m_out[0:feat, t * NT:t * NT + hi],
                                pm[:, 0:hi], cbias)

              def gru(dd, m_bf, wih, whh, bR, bZ, bI, bH, tag="gru",
                      exp=None):
                with sc(tag):
                  for t, sl, hi in mov_chunks(NL):
                      pR = p1.tile([dd, NT], F32, tag="pgR")
                      pZ = p1.tile([dd, NT], F32, tag="pgZ")
                      pI = p1.tile([dd, NT], F32, tag="pgI")
                      pH = p1.tile([dd, NT], F32, tag="pgH")
                      nc.tensor.matmul(pR[:, 0:hi], wih(0, dd), m_bf[0:dd, sl],
                                       start=True, stop=False)
                      nc.tensor.matmul(pR[:, 0:hi], whh(0, dd), A_hb[0:dd, sl],
                                       start=False, stop=True)
                      nc.tensor.matmul(pZ[:, 0:hi], wih(1, dd),
                                       m_bf[0:dd, sl], start=True, stop=False)
                      nc.tensor.matmul(pZ[:, 0:hi], whh(1, dd),
                                       A_hb[0:dd, sl], start=False, stop=True)
                      nc.tensor.matmul(pI[:, 0:hi], wih(2, dd), m_bf[0:dd, sl],
                                       start=True, stop=True)
                      nc.tensor.matmul(pH[:, 0:hi], whh(2, dd), A_hb[0:dd, sl],
                                       start=True, stop=True)
                      rs = wp.tile([dd, NT], BF16, tag="grs")
                      zs = wp.tile([dd, NT], BF16, tag="gzs")
                      nc.scalar.activation(rs[:, 0:hi], pR[:, 0:hi], AF.Sigmoid,
                                           bias=bR)
                      nc.scalar.activation(zs[:, 0:hi], pZ[:, 0:hi], AF.Sigmoid,
                                           bias=bZ)
                      hs = wp.tile([dd, NT], BF16, tag="ghs")
                      nc.scalar.activation(hs[:, 0:hi], pH[:, 0:hi], AF.Identity,
                                           bias=bH)
                      pIb = wp.tile([dd, NT], BF16, tag="gpib")
                      nc.scalar.activation(pIb[:, 0:hi], pI[:, 0:hi],
                                           AF.Identity, bias=bI)
                      t1 = wp.tile([dd, NT], BF16, tag="gt1")
                      nc.vector.tensor_tensor(t1[:, 0:hi], rs[:, 0:hi],
                                              hs[:, 0:hi], op=ALU.mult)
                      nc.vector.tensor_tensor(t1[:, 0:hi], t1[:, 0:hi],
                                              pIb[:, 0:hi], op=ALU.add)
                      nt_ = wp.tile([dd, NT], BF16, tag="grs")
                      nc.scalar.activation(nt_[:, 0:hi], t1[:, 0:hi], AF.Tanh)
                      hm = wp.tile([dd, NT], BF16, tag="gt1")
                      nc.vector.tensor_tensor(hm[:, 0:hi], A_hb[0:dd, sl],
                                              nt_[:, 0:hi], op=ALU.subtract)
                      nc.vector.tensor_tensor(hm[:, 0:hi], hm[:, 0:hi],
                                              zs[:, 0:hi], op=ALU.mult)
                      nc.vector.tensor_tensor(A_hb[0:dd, sl], hm[:, 0:hi],
                                              nt_[:, 0:hi], op=ALU.add)
                      if exp is not None:
                          for j in range(4 * t, min(4 * t + 4, NJ)):
                              export_chunk(exp[0], A_hb, j)
                  if exp is not None:
                      export_fin(exp[1], exp[2])

              # ---------------- stage 1
              H1s = [H1a, H1b]
              if "noexp" not in ABL:
                  export_state(64, H1_loc, H1s[0], A_hb, tag="exp1_0")

              # stage1 edge constants: ea1 + expanded chunks (fills AG wait)
              with sc("ea1"):
               if "noea" not in ABL:
                ea1a = wp.tile([20, EP1], BF16, tag="ea1a", bufs=1)
                nc.sync.dma_start(ea1a[:], eaT1[:])
                for j, sl, w in mov_chunks(EP1):
                  p = p2.tile([12, NT], F32, tag="p512")
                  nc.tensor.matmul(p[:, 0:w], cb("leWt"), ea1a[:, sl],
                                   start=True, stop=True)
                  ea1c = wp.tile([12, NT], BF16, tag="ea1c")
                  lrelu_act(ea1c[:, 0:w], p[:, 0:w], 0.0)
                  for cc in range(6):
                      pe = p2.tile([128, NT], F32, tag="p512")
                      nc.tensor.matmul(pe[:, 0:w], cb("REP1", cc * 128, 128),
                                       ea1c[:, 0:w], start=True, stop=True)
                      nc.scalar.activation(A_eax[:, cc * EP1 + sl.start:
                                                 cc * EP1 + sl.stop],
                                           pe[:, 0:w], AF.Identity)

              G1c = lambda kc, feat: cb("G1", kc * 64, 64)
              G2c = lambda kc, feat: cb("G2", kc * 128, 128)
              wih1f = lambda gt, dd: cb("wih1", gt * 64, 64)
              whh1f = lambda gt, dd: cb("whh1", gt * 64, 64)
              wih2f = lambda gt, dd: cb("wih2", gt * 128, 128)
              whh2f = lambda gt, dd: cb("whh2", gt * 128, 128)

              for it in range(2):
                  if "nostage1" not in ABL and "nomp" not in ABL:
                      msg_pass(H1s[it], ci("gidx1"), EP1, 6, G1c, None, S1,
                               J1, 64, A_m, cf("c1b"), tag=f"mp1_{it}")
                  if "nostage1" not in ABL and "nogru" not in ABL:
                      gru(64, A_m, wih1f, whh1f, cf("br1"), cf("bz1"),
                          cf("bin1"), cf("bhn1"), tag=f"gru1_{it}",
                          exp=((64, H1_loc, H1s[1]) if it == 0
                               and "noexp" not in ABL else None))

              # ---------------- lin_covert (h: 64 -> 128 features)
              with sc("covert"):
                for j, sl, w in mov_chunks(NL):
                  p = p2.tile([128, NT], F32, tag="p512")
                  nc.tensor.matmul(p[:, 0:w], cb("W1cT"), A_hb[0:64, sl],
                                   start=True, stop=True)
                  lrelu_act(A_m[:, sl], p[:, 0:w], cf("b1c"))
                for j, sl, w in mov_chunks(NL):
                  p = p2.tile([128, NT], F32, tag="p512")
                  nc.tensor.matmul(p[:, 0:w], cb("W2cT"), A_m[:, sl],
                                   start=True, stop=True)
                  lrelu_act(A_hb[:, sl], p[:, 0:w], cf("b2c"))

              # ---------------- stage 2
              H2s = [H2a, H2b, H2c]
              if "noexp" not in ABL:
                  export_state(128, H2_loc, H2s[0], A_hb, tag="exp2_0")

              # stage2 edge constants + resident S3 (fills AG wait)
              with sc("ea3"):
               if "noea" not in ABL:
                nc.sync.dma_start(A_s3[:], S3[:])
                ea3a = wp.tile([8, EP3], BF16, tag="ea3a", bufs=1)
                nc.sync.dma_start(ea3a[:], eaT3[:])
                for j, sl, w in mov_chunks(EP3):
                  for k in range(8):
                      p = p2.tile([128, NT], F32, tag="p512")
                      nc.tensor.matmul(p[:, 0:w], cb("REP2", k * 128, 128),
                                       ea3a[:, sl], start=True, stop=True)
                      nc.scalar.activation(A_eax[:, k * EP3 + sl.start:
                                                 k * EP3 + sl.stop],
                                           p[:, 0:w], AF.Identity)

              for it in range(2):
                  if "nostage2" not in ABL and "nomp" not in ABL:
                      msg_pass(H2s[it], ci("gidx3"), EP3, 8, G2c, A_s3, S3,
                               J3, 128, A_m, cf("c2b"), tag=f"mp2_{it}")
                  if "nostage2" not in ABL and "nogru" not in ABL:
                      gru(128, A_m, wih2f, whh2f, cf("br2"), cf("bz2"),
                          cf("bin2"), cf("bhn2"), tag=f"gru2_{it}",
                          exp=((128, H2_loc, H2s[it + 1])
                               if "noexp" not in ABL else None))

              # ---------------- final readout
              with sc("final"):
                t0 = fp.tile([128, 1, EF], BF16, tag="t0")
                t1_ = fp.tile([128, 1, EF], BF16, tag="t1")
                ea3l = cf("ea3locT")
                pwS = fp.tile([128, 3 * EF], F32, tag="pwS")
                for bi in range(3):
                  pw = p2.tile([128, EF], F32, tag="p512")
                  nc.tensor.matmul(pw[:], cf("lwWt", bi * 128, 128),
                                   ea3l, start=True, stop=True)
                  nc.scalar.activation(pwS[:, bi * EF:(bi + 1) * EF], pw[:],
                                       AF.Identity)
                nc.gpsimd.dma_gather(
                    t0[:], H2c[:], ci("gf0"), EF, EF, 128,
                    transpose=True, single_packet=False)
                nc.gpsimd.dma_gather(
                    t1_[:], H2c[:], ci("gf1"), EF, EF, 128,
                    transpose=True, single_packet=False)
                a0 = t0[:].rearrange("p one e -> p (one e)")
                a1 = t1_[:].rearrange("p one e -> p (one e)")
                fB0 = fp.tile([128, EF], F32, tag="fB0")
                fB1 = fp.tile([128, EF], F32, tag="fB1")
                fB2 = fp.tile([128, EF], F32, tag="fB2")
                fB = [fB0, fB1, fB2]
                nc.vector.tensor_tensor(fB[0][:], a0, a1, op=ALU.add)
                nc.vector.tensor_tensor(fB[1][:], a0, a1, op=ALU.mult)
                nc.vector.tensor_tensor(fB[2][:], a0, a1, op=ALU.subtract)
                nc.vector.tensor_tensor(fB[2][:], fB[2][:], fB[2][:],
                                        op=ALU.mult)
                pacc = p2.tile([1, EF], F32, tag="p512")
                for bi in range(3):
                  nc.vector.tensor_tensor(fB[bi][:], fB[bi][:],
                                          pwS[:, bi * EF:(bi + 1) * EF],
                                          op=ALU.mult)
                  nc.tensor.matmul(pacc[:], c_ones[:], fB[bi][:],
                                   start=(bi == 0), stop=False)
                nc.tensor.matmul(pacc[:], cf("lbp"), ea3l, start=False,
                                 stop=True)
                ot = fp.tile([1, EF], F32, tag="ot")
                nc.vector.tensor_copy(ot[:], pacc[:])
                nc.sync.dma_start(out_f[:], ot[:])

            for _rep in range(reps):
                one_pass()

    nc.compile()
    return nc


_CACHE = {}
_PREP_CACHE = {}


def kernel(**inputs):
    import hashlib
    hk = hashlib.sha1()
    for k in sorted(inputs):
        hk.update(k.encode())
        hk.update(np.ascontiguousarray(inputs[k]).tobytes())
    key = hk.hexdigest()
    if key not in _PREP_CACHE:
        _PREP_CACHE.clear()
        _PREP_CACHE[key] = _host_prep(inputs)
    static, in_maps = _PREP_CACHE[key]
    if static not in _CACHE:
        _CACHE[static] = _build(*static)
    nc = _CACHE[static]
    res = run_bass_kernel_spmd(nc, in_maps, list(range(C))).results
    return np.concatenate(
        [res[c]["out_f"][0, :E3 // C] for c in range(C)]).astype(np.float32)

